# revision 19
# baseline (speedup 1.0000x reference)
"""Trainium2 Bass kernel for nn_BQuantConv1d_simple.

Math: out[t, n] = sum_k (x2 @ binary[k])[t, n] * scale[k, 0, n] + bias[n]
with x2 = x.reshape(T, M).  scale has no m/t dependence, so it folds:

    W[m, n] = sum_k binary[k, m, n] * scale[k, 0, n]
    out     = x2 @ W + bias

which cuts the tensor-engine work 8x versus the unfolded form.

Two SPMD launches across the 8 NeuronCores:

  L1 (bit-sharded fold): core c computes Wc = binary[c] * scale[c] on the
     DVE.  The host sums the 8 partials in fp32 — the standard unshard step
     for a reduction-sharded computation.

  L2 (token-sharded matmul): core c computes out[tc] = x2[tc] @ W + bias on
     the tensor engine in fp16 (fp32 PSUM accumulation).  x is fed
     pre-transposed (m on partitions) since the PE contracts the partition
     axis of both operands.
"""

import numpy as np

import concourse.bass as bass
import concourse.mybir as mybir
import concourse.tile as tile
from concourse.bass_utils import run_bass_kernel_spmd

F16 = mybir.dt.float16
F32 = mybir.dt.float32

K, M, N = 8, 1024, 1024
B_, S_ = 4, 2048
T = B_ * S_            # 8192 tokens
NCORES = 8
TPC = T // NCORES      # 1024 tokens per core
P = 128                # partitions

_nc_cache = {}


def _legalize_sync_waits(nc):
    """This container's walrus build only accepts ONE sync-wait command per
    instruction (setupSyncWait in CoreV3GenImpl rejects more).  Tile emits
    up to 4.  Split the extras into single-wait NoOps placed immediately
    before the instruction on the same engine — the sequencer executes them
    in order, so the semantics are identical."""
    cnt = 0
    for fn in nc.m.functions:
        for blk in fn.blocks:
            insts = list(blk.instructions)
            out = []
            for inst in insts:
                si = inst.sync_info
                if si is not None and si.on_wait and len(si.on_wait) > 1:
                    waits = list(si.on_wait)
                    for w in waits[:-1]:
                        nop = mybir.InstNoOp(
                            name=f"legalize_wait_{cnt}", ins=[], outs=[])
                        cnt += 1
                        nop.engine = inst.engine
                        nop.sync_info = mybir.SyncInfo(on_wait=[w], on_update=[])
                        out.append(nop)
                    inst.sync_info = mybir.SyncInfo(
                        on_wait=[waits[-1]], on_update=list(si.on_update or []))
                out.append(inst)
            blk.instructions = out
    return nc


def _build_l1():
    """Per-core: w_part = b_in * s_in (per-output-channel scale, broadcast
    over the 128 partitions host-side).

    b is moved in 4 chunks of [128, 2048] (0.5 MB) to amortize the ~600ns
    HWDGE issue cost; every tile is resident (no slot-reuse waits)."""
    nc = bass.Bass("TRN2", num_devices=NCORES, enable_asserts=False)
    b_in = nc.dram_tensor("b_in", [M, N], F16, kind="ExternalInput")
    s_in = nc.dram_tensor("s_in", [P, N], F16, kind="ExternalInput")
    w_out = nc.dram_tensor("w_part", [M, N], F16, kind="ExternalOutput")

    CH = 4                       # chunks
    A = M // P // CH             # m-tiles per chunk (2)
    b_view = b_in.rearrange("(c a p) n -> c p a n", c=CH, a=A, p=P)
    w_view = w_out.rearrange("(c a p) n -> c p a n", c=CH, a=A, p=P)

    with tile.TileContext(nc) as tc:
        with tc.tile_pool(name="work", bufs=1) as pool:
            # All loads first: the SP sequencer is in-order, so a store that
            # waits on compute must not sit ahead of an independent load.
            s_sb = pool.tile([P, N], F16, tag="s")
            nc.sync.dma_start(s_sb[:], s_in[:])
            b_sbs = []
            for ci in range(CH):
                b_sb = pool.tile([P, A, N], F16, tag=f"b{ci}", name=f"b{ci}")
                nc.sync.dma_start(b_sb[:], b_view[ci])
                b_sbs.append(b_sb)
            for ci in range(CH):
                w_sb = pool.tile([P, A, N], F16, tag=f"w{ci}", name=f"w{ci}")
                for a in range(A):
                    nc.vector.tensor_mul(w_sb[:, a, :], b_sbs[ci][:, a, :], s_sb[:])
                nc.sync.dma_start(w_view[ci], w_sb[:])
    return nc


def _build_l2():
    """Per-core: out = x2[tc] @ W + bias (token shard).

    W and xT are fed as ONE fused input wx [M, N + TPC] so each m-block
    arrives in a single 0.5 MB DMA.  Loop is mb-outer over 4 token-tiles
    at a time (8 PSUM banks = 4 tt x 2 nb accumulation groups), so the
    matmul stream starts as soon as wx[0] lands and is never load-starved."""
    nc = bass.Bass("TRN2", num_devices=NCORES, enable_asserts=False)
    wx_in = nc.dram_tensor("wx_in", [M, N + TPC], F16, kind="ExternalInput")
    bias_in = nc.dram_tensor("bias_in", [P, N], F32, kind="ExternalInput")
    out = nc.dram_tensor("out", [TPC, N], F32, kind="ExternalOutput")

    MB = M // P        # 8 contraction tiles
    TT = TPC // P      # 8 token tiles
    NBW = 512          # one PSUM bank of fp32
    NB = N // NBW      # 2 n blocks
    TG = 4             # token-tiles processed per group (TG*NB = 8 banks)

    with tile.TileContext(nc) as tc:
        with (
            tc.tile_pool(name="const", bufs=1) as cpool,
            tc.tile_pool(name="psum", bufs=1, space=bass.MemorySpace.PSUM) as ppool,
            tc.tile_pool(name="out", bufs=4) as opool,
        ):
            # PE warmup: the HAM clock gate needs ~3.4us of sustained PE
            # activity before it lifts the 1.2GHz -> 2.4GHz throttle.  Run
            # dummy matmuls on a zeroed scratch tile while the first wx
            # DMAs are in flight so the real matmul stream runs warm.
            # The warm psum uses the bank tag whose first real use comes
            # latest, so warmup never delays a real accumulation group.
            warm_sb = cpool.tile([P, NBW], F16, tag="warm")
            nc.gpsimd.memset(warm_sb[:], 0.0)
            warm_ps = ppool.tile([P, NBW], F32, tag=f"ps_{TG-1}_{NB-1}",
                                 name="warm_ps")
            # 11 warm matmuls ~ 4.5us at the cold clock: enough for one full
            # 3.4us HAM busy-window AND to bridge the gap until wx0 lands,
            # so the PE never idles (an idle gap before the un-throttle
            # restarts the busy-window accounting).
            for i in range(11):
                nc.tensor.matmul(
                    warm_ps[:], warm_sb[:, :P], warm_sb[:],
                    start=True, stop=True,
                )

            wx_sb = []
            for mb in range(MB):
                wx_t = cpool.tile([P, N + TPC], F16, tag=f"wx{mb}",
                                  name=f"wx{mb}")
                nc.sync.dma_start(wx_t[:], wx_in[mb * P:(mb + 1) * P, :])
                wx_sb.append(wx_t)
            bias_sb = cpool.tile([P, N], F32, tag="bias")
            nc.sync.dma_start(bias_sb[:], bias_in[:])

            # First group: 4 token-tiles (8 banks) so early matmul demand
            # stays below the streaming-load rate.  Then single-tile groups
            # (2 banks each) so the final bias-add/store tail is short.
            groups = [list(range(TG))] + [[tt] for tt in range(TG, TT)]
            for grp in groups:
                psums = {}
                for tt in grp:
                    for nb in range(NB):
                        psums[(tt, nb)] = ppool.tile(
                            [P, NBW], F32, tag=f"ps_{tt % TG}_{nb}",
                            name=f"ps{tt}_{nb}")
                for mb in range(MB):
                    for tt in grp:
                        lhsT = wx_sb[mb][:, N + tt * P:N + (tt + 1) * P]
                        for nb in range(NB):
                            nc.tensor.matmul(
                                psums[(tt, nb)][:],
                                lhsT,
                                wx_sb[mb][:, nb * NBW:(nb + 1) * NBW],
                                start=(mb == 0),
                                stop=(mb == MB - 1),
                            )
                for tt in grp:
                    for nb in range(NB):
                        nsl = slice(nb * NBW, (nb + 1) * NBW)
                        o_t = opool.tile([P, NBW], F32, tag="o",
                                         name=f"o{tt}_{nb}")
                        nc.vector.tensor_add(
                            o_t[:], psums[(tt, nb)][:], bias_sb[:, nsl])
                        nc.sync.dma_start(out[tt * P:(tt + 1) * P, nsl], o_t[:])
    return nc


def _get_nc(name):
    if name not in _nc_cache:
        _nc_cache[name] = _legalize_sync_waits(
            {"l1": _build_l1, "l2": _build_l2}[name]())
    return _nc_cache[name]


def run_sharded(x, binary, scale, bias, trace=False):
    """Returns (out_full, [l1_results, l2_results])."""
    x = np.asarray(x, dtype=np.float32)
    binary = np.asarray(binary, dtype=np.float32)
    scale = np.asarray(scale, dtype=np.float32)
    bias = np.asarray(bias, dtype=np.float32)

    core_ids = list(range(NCORES))

    # ---- L1: bit-sharded scale fold -------------------------------------
    in_maps1 = []
    for c in range(NCORES):
        in_maps1.append({
            "b_in": binary[c].astype(np.float16),          # +/-1: lossless
            "s_in": np.ascontiguousarray(
                np.broadcast_to(scale[c, 0], (P, N))).astype(np.float16),
        })
    r1 = run_bass_kernel_spmd(_get_nc("l1"), in_maps1, core_ids, trace=trace)

    w32 = np.zeros((M, N), dtype=np.float32)
    for c in range(NCORES):
        w32 += r1.results[c]["w_part"].astype(np.float32)
    w16 = w32.astype(np.float16)

    # ---- L2: token-sharded matmul ---------------------------------------
    x2 = x.reshape(T, M)
    bias_b = np.ascontiguousarray(np.broadcast_to(bias, (P, N)))
    in_maps2 = []
    for c in range(NCORES):
        xt = x2[c * TPC:(c + 1) * TPC].T.astype(np.float16)
        wx = np.concatenate([w16, xt], axis=1)          # [M, N + TPC]
        in_maps2.append({"wx_in": np.ascontiguousarray(wx), "bias_in": bias_b})
    r2 = run_bass_kernel_spmd(_get_nc("l2"), in_maps2, core_ids, trace=trace)

    out = np.concatenate([r2.results[c]["out"] for c in range(NCORES)], axis=0)
    return out.reshape(B_, S_, N), [r1, r2]


def kernel(x, binary, scale, bias):
    out, _ = run_sharded(x, binary, scale, bias, trace=False)
    return out
